# revision 12
# baseline (speedup 1.0000x reference)
"""Multi-head self-attention on 8 TRN2 NeuronCores.

Problem: x(4,2048,1024), Wq(8,1024,128), Wk/Wv(1024,128), Wo(1024,1024) fp32.
out = softmax(Q K^T / sqrt(128)) V -> concat heads -> @ Wo.

Sharding: (batch, query-half) across 8 cores — core c handles batch c//2,
query rows [(c%2)*1024, (c%2)*1024+1024). K/V cover the full sequence of the
batch, so each core computes them locally from its x slice; no collectives.

Numerics: scores have std ~1024 and softmax is near-one-hot, so the
x->Q/K->scores chain needs ~fp32 precision. bf16 matmuls with hi/lo split
operands ("split3": Ah*Bh + Ah*Bl + Al*Bh, fp32 PSUM accumulation) give
~5e-6 relative matmul error at 3 cycles/row (native fp32 is 4). The x and
weight splits are precomputed on the host. V/ctx/Wo paths are plain bf16.

Layouts (partition dim first):
  xT (E,S) host-transposed; K^T (O,S) = sum_e Wk[e].T-stationary @ xT[e];
  Q_h^T (O,Sq) likewise (Wq pre-scaled by 1/sqrt(O) on host);
  scores tile (128q, 2048s) = Q^T-slice-stationary @ K^T-moving, fp32 PSUM,
  bank-chunk-major so each 512-col bank finishes early;
  softmax per q-row: per-bank DVE reduce_max -> combine(negate) -> per-bank
  ACT exp(bias=-max, accum_out=den chunk) -> den sum -> 1/den -> DVE scale;
  P transposed 128x128 via PE right after each q-tile (PE gap filler);
  ctx^T (O,Sq) = V-stationary @ P^T-moving; out (Sq,E) = ctx-slices-stationary
  @ Wo-moving (natural output layout).
"""
import numpy as np
import ml_dtypes

B, S, E, H, O = 4, 2048, 1024, 8, 128
SQ = S // 2          # query rows per core
NCORES = 8
ET = E // 128        # 8 e-tiles
ST = S // 128        # 16 s-tiles
QT = SQ // 128       # 8 q-tiles
NB = S // 512        # 4 score banks per q-tile
EC = E // 512        # 2 out-proj column chunks

_compiled = None     # cache so repeated kernel() calls skip rebuild


def _build():
    import concourse.bass as bass
    import concourse.mybir as mybir
    import concourse.tile as tile
    from concourse import bacc
    from concourse.masks import make_identity

    F32 = mybir.dt.float32
    BF16 = mybir.dt.bfloat16
    PS = bass.MemorySpace.PSUM
    EXP = mybir.ActivationFunctionType.Exp

    nc = bacc.Bacc("TRN2", target_bir_lowering=False, debug=False,
                   enable_asserts=True)

    d_xkvh = nc.dram_tensor("xkvh", (E, S), BF16, kind="ExternalInput").ap()
    d_xkvl = nc.dram_tensor("xkvl", (E, S), BF16, kind="ExternalInput").ap()
    d_xqh = nc.dram_tensor("xqh", (E, SQ), BF16, kind="ExternalInput").ap()
    d_xql = nc.dram_tensor("xql", (E, SQ), BF16, kind="ExternalInput").ap()
    d_wqh = nc.dram_tensor("wqh", (H, E, O), BF16, kind="ExternalInput").ap()
    d_wql = nc.dram_tensor("wql", (H, E, O), BF16, kind="ExternalInput").ap()
    d_wkh = nc.dram_tensor("wkh", (E, O), BF16, kind="ExternalInput").ap()
    d_wkl = nc.dram_tensor("wkl", (E, O), BF16, kind="ExternalInput").ap()
    d_wvh = nc.dram_tensor("wvh", (E, O), BF16, kind="ExternalInput").ap()
    d_woh = nc.dram_tensor("woh", (H * O, E), BF16, kind="ExternalInput").ap()
    d_out = nc.dram_tensor("out", (SQ, E), F32, kind="ExternalOutput").ap()

    with tile.TileContext(nc) as tc:
        with (
            tc.tile_pool(name="persist", bufs=1) as persist,
            tc.tile_pool(name="tiny", bufs=24) as tiny,
        ):
            ident = persist.tile([128, 128], BF16, tag="ident")
            make_identity(nc, ident[:])

            wo_sb = persist.tile([128, H, E], BF16, tag="wo")
            nc.sync.dma_start(wo_sb[:], d_woh.rearrange("(h p) e -> p h e", p=128))

            kth = persist.tile([128, S], BF16, tag="kth")
            ktl = persist.tile([128, S], BF16, tag="ktl")
            qth = persist.tile([128, H, SQ], BF16, tag="qth")
            qtl = persist.tile([128, H, SQ], BF16, tag="qtl")
            v_sb = persist.tile([128, ST, O], BF16, tag="v")

            # ---------------- prologue: K^T, V, Q^T projections ----------
            with tc.tile_pool(name="xp", bufs=1) as xp:
                wkh = xp.tile([128, ET, O], BF16, tag="wkh")
                wkl = xp.tile([128, ET, O], BF16, tag="wkl")
                wvh = xp.tile([128, ET, O], BF16, tag="wvh")
                nc.sync.dma_start(wkh[:], d_wkh.rearrange("(t p) o -> p t o", p=128))
                nc.sync.dma_start(wkl[:], d_wkl.rearrange("(t p) o -> p t o", p=128))
                nc.sync.dma_start(wvh[:], d_wvh.rearrange("(t p) o -> p t o", p=128))
                xkvh = xp.tile([128, ET, S], BF16, tag="xkvh")
                xkvl = xp.tile([128, ET, S], BF16, tag="xkvl")
                for e in range(ET):
                    nc.sync.dma_start(xkvh[:, e, :], d_xkvh[e * 128:(e + 1) * 128, :])
                    nc.sync.dma_start(xkvl[:, e, :], d_xkvl[e * 128:(e + 1) * 128, :])

                # K^T (128o x 2048s), split3 accumulation over e
                with tc.tile_pool(name="ktp", bufs=1, space=PS) as ktp:
                    kt_ps = ktp.tile([128, S], F32, tag="kt")
                    for e in range(ET):
                        for ti, (w, xx) in enumerate(
                            ((wkh, xkvh), (wkh, xkvl), (wkl, xkvh))
                        ):
                            for c in range(NB):
                                nc.tensor.matmul(
                                    kt_ps[:, c * 512:(c + 1) * 512],
                                    w[:, e, :],
                                    xx[:, e, c * 512:(c + 1) * 512],
                                    start=(e == 0 and ti == 0),
                                    stop=(e == ET - 1 and ti == 2),
                                )
                    nc.scalar.copy(kth[:], kt_ps[:])
                    nc.vector.tensor_sub(ktl[:], kt_ps[:], kth[:])

                with (
                    tc.tile_pool(name="vp", bufs=2, space=PS) as vp,
                    tc.tile_pool(name="qp", bufs=2, space=PS) as qp,
                ):
                    # V (s-part tiles), plain bf16
                    for st in range(ST):
                        v_ps = vp.tile([128, O], F32, tag="vps")
                        for e in range(ET):
                            nc.tensor.matmul(
                                v_ps[:],
                                xkvh[:, e, st * 128:(st + 1) * 128],
                                wvh[:, e, :],
                                start=(e == 0),
                                stop=(e == ET - 1),
                            )
                        nc.vector.tensor_copy(v_sb[:, st, :], v_ps[:])

                    # Q^T per head (Wq pre-scaled by 1/sqrt(O) on host)
                    xqh = xp.tile([128, ET, SQ], BF16, tag="xqh")
                    xql = xp.tile([128, ET, SQ], BF16, tag="xql")
                    for e in range(ET):
                        nc.sync.dma_start(xqh[:, e, :], d_xqh[e * 128:(e + 1) * 128, :])
                        nc.sync.dma_start(xql[:, e, :], d_xql[e * 128:(e + 1) * 128, :])
                    wqh = xp.tile([128, H, ET, O], BF16, tag="wqh")
                    wql = xp.tile([128, H, ET, O], BF16, tag="wql")
                    nc.sync.dma_start(
                        wqh[:], d_wqh.rearrange("h (t p) o -> p h t o", p=128))
                    nc.sync.dma_start(
                        wql[:], d_wql.rearrange("h (t p) o -> p h t o", p=128))

                    for h in range(H):
                        q_ps = qp.tile([128, SQ], F32, tag="qtps")
                        for e in range(ET):
                            for ti, (w, xx) in enumerate(
                                ((wqh, xqh), (wqh, xql), (wql, xqh))
                            ):
                                for c in range(SQ // 512):
                                    nc.tensor.matmul(
                                        q_ps[:, c * 512:(c + 1) * 512],
                                        w[:, h, e, :],
                                        xx[:, e, c * 512:(c + 1) * 512],
                                        start=(e == 0 and ti == 0),
                                        stop=(e == ET - 1 and ti == 2),
                                    )
                        nc.scalar.copy(qth[:, h, :], q_ps[:])
                        nc.vector.tensor_sub(qtl[:, h, :], q_ps[:], qth[:, h, :])

            # ---------------- main: per-head attention ------------------
            # PSUM budget (8 banks): "acc1024" 2-bank tiles x3 bufs shared by
            # score-halves, ctx and out accumulators (6 banks) + one 2-bank
            # transpose staging tile.  Score halves cycle through 3 slots so
            # the next q-tile's matmuls never wait on this one's softmax.
            with (
                tc.tile_pool(name="p_pool", bufs=4) as p_pool,
                tc.tile_pool(name="pt_pool", bufs=1) as pt_pool,
                tc.tile_pool(name="ctx_pool", bufs=H) as ctx_pool,
                tc.tile_pool(name="acc_ps", bufs=3, space=PS) as acc_psp,
                tc.tile_pool(name="pt_ps", bufs=1, space=PS) as pt_psp,
                tc.tile_pool(name="o_sb", bufs=2) as o_sbp,
            ):
                HS = S // 2  # 1024-wide score half
                ctxs = []
                for h in range(H):
                    pt_h = pt_pool.tile([128, ST, SQ], BF16, tag="pt")
                    for qt in range(QT):
                        negmax4 = tiny.tile([128, NB], F32, tag="negmax4")
                        den2 = tiny.tile([128, 2], F32, tag="den2")
                        halves = []
                        for sh in range(2):
                            s_ps = acc_psp.tile([128, HS], F32, tag="acc1024")
                            halves.append(s_ps)
                            for ti, (qq, kk) in enumerate(
                                ((qth, kth), (qth, ktl), (qtl, kth))
                            ):
                                for c in range(2):
                                    nc.tensor.matmul(
                                        s_ps[:, c * 512:(c + 1) * 512],
                                        qq[:, h, qt * 128:(qt + 1) * 128],
                                        kk[:, sh * HS + c * 512:
                                           sh * HS + (c + 1) * 512],
                                        start=(ti == 0),
                                        stop=(ti == 2),
                                    )
                            for c in range(2):
                                nc.vector.reduce_max(
                                    out=negmax4[:, sh * 2 + c:sh * 2 + c + 1],
                                    in_=s_ps[:, c * 512:(c + 1) * 512],
                                    axis=mybir.AxisListType.X,
                                )
                        negmax = tiny.tile([128, 1], F32, tag="negmax")
                        nc.vector.reduce_max(
                            out=negmax[:], in_=negmax4[:],
                            axis=mybir.AxisListType.X, negate=True,
                        )
                        p_qt = p_pool.tile([128, S], BF16, tag="p")
                        for sh in range(2):
                            nc.scalar.activation(
                                p_qt[:, sh * HS:(sh + 1) * HS],
                                halves[sh][:],
                                EXP, bias=negmax[:], scale=1.0,
                                accum_out=den2[:, sh:sh + 1],
                            )
                        den = tiny.tile([128, 1], F32, tag="den")
                        nc.vector.reduce_sum(
                            out=den[:], in_=den2[:], axis=mybir.AxisListType.X)
                        invden = tiny.tile([128, 1], F32, tag="invden")
                        nc.vector.reciprocal(invden[:], den[:])
                        # the big P normalization runs on the otherwise-idle
                        # GpSimd so the DVE stream stays clear for PSUM reduces
                        nc.gpsimd.tensor_scalar_mul(p_qt[:], p_qt[:], invden[:])

                        # transpose this q-tile's P right away: PE filler work
                        pt_ps = pt_psp.tile([128, ST, 128], BF16, tag="ptps")
                        for st in range(ST):
                            nc.tensor.transpose(
                                pt_ps[:, st, :],
                                p_qt[:, st * 128:(st + 1) * 128],
                                ident[:],
                            )
                        cp = nc.scalar.copy if qt % 2 else nc.vector.tensor_copy
                        cp(pt_h[:, :, qt * 128:(qt + 1) * 128], pt_ps[:])

                    # ctx^T (o-part, q-free) accumulated over s-tiles
                    ct_ps = acc_psp.tile([128, SQ], F32, tag="acc1024")
                    for qc in range(SQ // 512):
                        for st in range(ST):
                            nc.tensor.matmul(
                                ct_ps[:, qc * 512:(qc + 1) * 512],
                                v_sb[:, st, :],
                                pt_h[:, st, qc * 512:(qc + 1) * 512],
                                start=(st == 0),
                                stop=(st == ST - 1),
                            )
                    ctx_h = ctx_pool.tile([128, SQ], BF16, tag="ctx")
                    nc.vector.tensor_copy(ctx_h[:], ct_ps[:])
                    ctxs.append(ctx_h)

                # ------- out (q-part, e-free) = sum_h ctx_h^T-slices @ Wo_h
                for qt in range(QT):
                    o_ps = acc_psp.tile([128, E], F32, tag="acc1024")
                    for h in range(H):
                        for ec in range(EC):
                            nc.tensor.matmul(
                                o_ps[:, ec * 512:(ec + 1) * 512],
                                ctxs[h][:, qt * 128:(qt + 1) * 128],
                                wo_sb[:, h, ec * 512:(ec + 1) * 512],
                                start=(h == 0),
                                stop=(h == H - 1),
                            )
                    o_sb = o_sbp.tile([128, E], F32, tag="osb")
                    nc.vector.tensor_copy(o_sb[:], o_ps[:])
                    nc.sync.dma_start(d_out[qt * 128:(qt + 1) * 128, :], o_sb[:])

    nc.compile()
    return nc


def _split(a):
    """fp32 -> (hi, lo) bf16 pair with hi + lo ~= a."""
    hi = a.astype(ml_dtypes.bfloat16)
    lo = (a - hi.astype(np.float32)).astype(ml_dtypes.bfloat16)
    return hi, lo


def kernel(x, Wq, Wk, Wv, Wo):
    global _compiled
    from concourse.bass_utils import run_bass_kernel_spmd

    if _compiled is None:
        _compiled = _build()
    nc = _compiled

    scale = np.float32(1.0 / np.sqrt(O))
    wqh, wql = _split(Wq.astype(np.float32) * scale)
    wkh, wkl = _split(Wk.astype(np.float32))
    wvh = Wv.astype(ml_dtypes.bfloat16)
    woh = Wo.astype(ml_dtypes.bfloat16)

    in_maps = []
    for c in range(NCORES):
        b, half = divmod(c, 2)
        xT = np.ascontiguousarray(x[b].T)          # (E, S) fp32
        xh, xl = _split(xT)
        in_maps.append({
            "xkvh": xh, "xkvl": xl,
            "xqh": np.ascontiguousarray(xh[:, half * SQ:(half + 1) * SQ]),
            "xql": np.ascontiguousarray(xl[:, half * SQ:(half + 1) * SQ]),
            "wqh": wqh, "wql": wql,
            "wkh": wkh, "wkl": wkl, "wvh": wvh, "woh": woh,
        })

    res = run_bass_kernel_spmd(nc, in_maps, core_ids=list(range(NCORES)))

    out = np.empty((B, S, E), dtype=np.float32)
    for c in range(NCORES):
        b, half = divmod(c, 2)
        out[b, half * SQ:(half + 1) * SQ, :] = res.results[c]["out"]
    return out


# revision 15
# speedup vs baseline: 3.3052x; 3.3052x over previous
"""Multi-head self-attention on 8 TRN2 NeuronCores.

Problem: x(4,2048,1024), Wq(8,1024,128), Wk/Wv(1024,128), Wo(1024,1024) fp32.
out = softmax(Q K^T / sqrt(128)) V -> concat heads -> @ Wo.

Sharding: (batch, query-half) across 8 cores — core c handles batch c//2,
query rows [(c%2)*1024, (c%2)*1024+1024). K/V cover the full sequence of the
batch, so each core computes them locally from its x slice; no collectives.

Numerics: scores have std ~1024 and softmax is near-one-hot, so the
x->Q/K->scores chain needs ~fp32 precision. bf16 matmuls with hi/lo split
operands ("split3": Ah*Bh + Ah*Bl + Al*Bh, fp32 PSUM accumulation) give
~5e-6 relative matmul error at 3 cycles/row (native fp32 is 4). The x and
weight splits are precomputed on the host. V/ctx/Wo paths are plain bf16.

Layouts (partition dim first):
  xT (E,S) host-transposed; K^T (O,S) = sum_e Wk[e].T-stationary @ xT[e];
  Q_h^T (O,Sq) likewise (Wq pre-scaled by 1/sqrt(O) on host);
  scores tile (128q, 2048s) = Q^T-slice-stationary @ K^T-moving, fp32 PSUM,
  bank-chunk-major so each 512-col bank finishes early;
  softmax per q-row: per-bank DVE reduce_max -> combine(negate) -> per-bank
  ACT exp(bias=-max, accum_out=den chunk) -> den sum -> 1/den -> DVE scale;
  P transposed 128x128 via PE right after each q-tile (PE gap filler);
  ctx^T (O,Sq) = V-stationary @ P^T-moving; out (Sq,E) = ctx-slices-stationary
  @ Wo-moving (natural output layout).
"""
import numpy as np
import ml_dtypes

B, S, E, H, O = 4, 2048, 1024, 8, 128
SQ = S // 2          # query rows per core
NCORES = 8
ET = E // 128        # 8 e-tiles
ST = S // 128        # 16 s-tiles
QT = SQ // 128       # 8 q-tiles
NB = S // 512        # 4 score banks per q-tile
EC = E // 512        # 2 out-proj column chunks

_compiled = None     # cache so repeated kernel() calls skip rebuild


def _build():
    import concourse.bass as bass
    import concourse.mybir as mybir
    import concourse.tile as tile
    from concourse import bacc
    from concourse.masks import make_identity

    F32 = mybir.dt.float32
    BF16 = mybir.dt.bfloat16
    PS = bass.MemorySpace.PSUM
    EXP = mybir.ActivationFunctionType.Exp

    nc = bacc.Bacc("TRN2", target_bir_lowering=False, debug=False,
                   enable_asserts=True)

    d_xkvh = nc.dram_tensor("xkvh", (E, S), BF16, kind="ExternalInput").ap()
    d_xkvl = nc.dram_tensor("xkvl", (E, S), BF16, kind="ExternalInput").ap()
    d_xqh = nc.dram_tensor("xqh", (E, SQ), BF16, kind="ExternalInput").ap()
    d_xql = nc.dram_tensor("xql", (E, SQ), BF16, kind="ExternalInput").ap()
    d_wqh = nc.dram_tensor("wqh", (H, E, O), BF16, kind="ExternalInput").ap()
    d_wql = nc.dram_tensor("wql", (H, E, O), BF16, kind="ExternalInput").ap()
    d_wkh = nc.dram_tensor("wkh", (E, O), BF16, kind="ExternalInput").ap()
    d_wkl = nc.dram_tensor("wkl", (E, O), BF16, kind="ExternalInput").ap()
    d_wvh = nc.dram_tensor("wvh", (E, O), BF16, kind="ExternalInput").ap()
    d_woh = nc.dram_tensor("woh", (H * O, E), BF16, kind="ExternalInput").ap()
    d_out = nc.dram_tensor("out", (SQ, E), F32, kind="ExternalOutput").ap()

    with tile.TileContext(nc) as tc:
        with (
            tc.tile_pool(name="persist", bufs=1) as persist,
            tc.tile_pool(name="tiny", bufs=24) as tiny,
        ):
            ident = persist.tile([128, 128], BF16, tag="ident")
            make_identity(nc, ident[:])

            wo_sb = persist.tile([128, H, E], BF16, tag="wo")
            nc.sync.dma_start(wo_sb[:], d_woh.rearrange("(h p) e -> p h e", p=128))

            kth = persist.tile([128, S], BF16, tag="kth")
            ktl = persist.tile([128, S], BF16, tag="ktl")
            qth = persist.tile([128, H, SQ], BF16, tag="qth")
            qtl = persist.tile([128, H, SQ], BF16, tag="qtl")
            v_sb = persist.tile([128, ST, O], BF16, tag="v")

            # ---------------- prologue: K^T, V, Q^T projections ----------
            with tc.tile_pool(name="xp", bufs=1) as xp:
                wkh = xp.tile([128, ET, O], BF16, tag="wkh")
                wkl = xp.tile([128, ET, O], BF16, tag="wkl")
                wvh = xp.tile([128, ET, O], BF16, tag="wvh")
                nc.sync.dma_start(wkh[:], d_wkh.rearrange("(t p) o -> p t o", p=128))
                nc.sync.dma_start(wkl[:], d_wkl.rearrange("(t p) o -> p t o", p=128))
                nc.sync.dma_start(wvh[:], d_wvh.rearrange("(t p) o -> p t o", p=128))
                xkvh = xp.tile([128, ET, S], BF16, tag="xkvh")
                xkvl = xp.tile([128, ET, S], BF16, tag="xkvl")
                for e in range(ET):
                    nc.sync.dma_start(xkvh[:, e, :], d_xkvh[e * 128:(e + 1) * 128, :])
                    nc.sync.dma_start(xkvl[:, e, :], d_xkvl[e * 128:(e + 1) * 128, :])

                # K^T (128o x 2048s), split3 accumulation over e
                with tc.tile_pool(name="ktp", bufs=1, space=PS) as ktp:
                    kt_ps = ktp.tile([128, S], F32, tag="kt")
                    for e in range(ET):
                        for ti, (w, xx) in enumerate(
                            ((wkh, xkvh), (wkh, xkvl), (wkl, xkvh))
                        ):
                            for c in range(NB):
                                nc.tensor.matmul(
                                    kt_ps[:, c * 512:(c + 1) * 512],
                                    w[:, e, :],
                                    xx[:, e, c * 512:(c + 1) * 512],
                                    start=(e == 0 and ti == 0),
                                    stop=(e == ET - 1 and ti == 2),
                                )
                    nc.scalar.copy(kth[:], kt_ps[:])
                    nc.vector.tensor_sub(ktl[:], kt_ps[:], kth[:])

                with (
                    tc.tile_pool(name="vp", bufs=2, space=PS) as vp,
                    tc.tile_pool(name="qp", bufs=2, space=PS) as qp,
                ):
                    # V (s-part tiles), plain bf16
                    for st in range(ST):
                        v_ps = vp.tile([128, O], F32, tag="vps")
                        for e in range(ET):
                            nc.tensor.matmul(
                                v_ps[:],
                                xkvh[:, e, st * 128:(st + 1) * 128],
                                wvh[:, e, :],
                                start=(e == 0),
                                stop=(e == ET - 1),
                            )
                        nc.vector.tensor_copy(v_sb[:, st, :], v_ps[:])

                    # Q^T per head (Wq pre-scaled by 1/sqrt(O) on host)
                    xqh = xp.tile([128, ET, SQ], BF16, tag="xqh")
                    xql = xp.tile([128, ET, SQ], BF16, tag="xql")
                    for e in range(ET):
                        nc.sync.dma_start(xqh[:, e, :], d_xqh[e * 128:(e + 1) * 128, :])
                        nc.sync.dma_start(xql[:, e, :], d_xql[e * 128:(e + 1) * 128, :])
                    wqh = xp.tile([128, H, ET, O], BF16, tag="wqh")
                    wql = xp.tile([128, H, ET, O], BF16, tag="wql")
                    nc.sync.dma_start(
                        wqh[:], d_wqh.rearrange("h (t p) o -> p h t o", p=128))
                    nc.sync.dma_start(
                        wql[:], d_wql.rearrange("h (t p) o -> p h t o", p=128))

                    for h in range(H):
                        q_ps = qp.tile([128, SQ], F32, tag="qtps")
                        for e in range(ET):
                            for ti, (w, xx) in enumerate(
                                ((wqh, xqh), (wqh, xql), (wql, xqh))
                            ):
                                for c in range(SQ // 512):
                                    nc.tensor.matmul(
                                        q_ps[:, c * 512:(c + 1) * 512],
                                        w[:, h, e, :],
                                        xx[:, e, c * 512:(c + 1) * 512],
                                        start=(e == 0 and ti == 0),
                                        stop=(e == ET - 1 and ti == 2),
                                    )
                        nc.scalar.copy(qth[:, h, :], q_ps[:])
                        nc.vector.tensor_sub(qtl[:, h, :], q_ps[:], qth[:, h, :])

            # ---------------- main: per-head attention ------------------
            # PSUM budget (8 banks): "acc1024" 2-bank tiles x3 bufs shared by
            # score-halves, ctx and out accumulators (6 banks) + one 2-bank
            # transpose staging tile.  Score halves cycle through 3 slots so
            # the next q-tile's matmuls never wait on this one's softmax.
            with (
                tc.tile_pool(name="p_pool", bufs=4) as p_pool,
                tc.tile_pool(name="pt_pool", bufs=2) as pt_pool,
                tc.tile_pool(name="ctx_pool", bufs=H) as ctx_pool,
                tc.tile_pool(name="acc_ps", bufs=3, space=PS) as acc_psp,
                tc.tile_pool(name="pt_ps", bufs=1, space=PS) as pt_psp,
                tc.tile_pool(name="o_sb", bufs=2) as o_sbp,
            ):
                HS = S // 2  # 1024-wide score half
                ctxs = []
                for h in range(H):
                    pt_h = pt_pool.tile([128, ST, SQ], BF16, tag="pt")
                    for qt in range(QT):
                        negmax4 = tiny.tile([128, NB], F32, tag="negmax4")
                        den2 = tiny.tile([128, 2], F32, tag="den2")
                        halves = []
                        for sh in range(2):
                            s_ps = acc_psp.tile([128, HS], F32, tag="acc1024")
                            halves.append(s_ps)
                            for ti, (qq, kk) in enumerate(
                                ((qth, kth), (qth, ktl), (qtl, kth))
                            ):
                                for c in range(2):
                                    nc.tensor.matmul(
                                        s_ps[:, c * 512:(c + 1) * 512],
                                        qq[:, h, qt * 128:(qt + 1) * 128],
                                        kk[:, sh * HS + c * 512:
                                           sh * HS + (c + 1) * 512],
                                        start=(ti == 0),
                                        stop=(ti == 2),
                                    )
                            for c in range(2):
                                nc.vector.reduce_max(
                                    out=negmax4[:, sh * 2 + c:sh * 2 + c + 1],
                                    in_=s_ps[:, c * 512:(c + 1) * 512],
                                    axis=mybir.AxisListType.X,
                                )
                        negmax = tiny.tile([128, 1], F32, tag="negmax")
                        nc.vector.reduce_max(
                            out=negmax[:], in_=negmax4[:],
                            axis=mybir.AxisListType.X, negate=True,
                        )
                        p_qt = p_pool.tile([128, S], BF16, tag="p")
                        for sh in range(2):
                            nc.scalar.activation(
                                p_qt[:, sh * HS:(sh + 1) * HS],
                                halves[sh][:],
                                EXP, bias=negmax[:], scale=1.0,
                                accum_out=den2[:, sh:sh + 1],
                            )
                        den = tiny.tile([128, 1], F32, tag="den")
                        nc.vector.reduce_sum(
                            out=den[:], in_=den2[:], axis=mybir.AxisListType.X)
                        invden = tiny.tile([128, 1], F32, tag="invden")
                        nc.vector.reciprocal(invden[:], den[:])
                        nc.vector.tensor_scalar_mul(p_qt[:], p_qt[:], invden[:])

                        # transpose this q-tile's P right away: PE filler work
                        pt_ps = pt_psp.tile([128, ST, 128], BF16, tag="ptps")
                        for st in range(ST):
                            nc.tensor.transpose(
                                pt_ps[:, st, :],
                                p_qt[:, st * 128:(st + 1) * 128],
                                ident[:],
                            )
                        # on ACT: ready right after this qt's exps, so it never
                        # head-of-line-blocks the next qt's softmax on DVE
                        nc.scalar.copy(
                            pt_h[:, :, qt * 128:(qt + 1) * 128], pt_ps[:])

                    # ctx^T (o-part, q-free) accumulated over s-tiles
                    ct_ps = acc_psp.tile([128, SQ], F32, tag="acc1024")
                    for qc in range(SQ // 512):
                        for st in range(ST):
                            nc.tensor.matmul(
                                ct_ps[:, qc * 512:(qc + 1) * 512],
                                v_sb[:, st, :],
                                pt_h[:, st, qc * 512:(qc + 1) * 512],
                                start=(st == 0),
                                stop=(st == ST - 1),
                            )
                    ctx_h = ctx_pool.tile([128, SQ], BF16, tag="ctx")
                    nc.vector.tensor_copy(ctx_h[:], ct_ps[:])
                    ctxs.append(ctx_h)

                # ------- out (q-part, e-free) = sum_h ctx_h^T-slices @ Wo_h
                for qt in range(QT):
                    o_ps = acc_psp.tile([128, E], F32, tag="acc1024")
                    for h in range(H):
                        for ec in range(EC):
                            nc.tensor.matmul(
                                o_ps[:, ec * 512:(ec + 1) * 512],
                                ctxs[h][:, qt * 128:(qt + 1) * 128],
                                wo_sb[:, h, ec * 512:(ec + 1) * 512],
                                start=(h == 0),
                                stop=(h == H - 1),
                            )
                    o_sb = o_sbp.tile([128, E], F32, tag="osb")
                    nc.vector.tensor_copy(o_sb[:], o_ps[:])
                    nc.sync.dma_start(d_out[qt * 128:(qt + 1) * 128, :], o_sb[:])

    nc.compile()
    return nc


def _split(a):
    """fp32 -> (hi, lo) bf16 pair with hi + lo ~= a."""
    hi = a.astype(ml_dtypes.bfloat16)
    lo = (a - hi.astype(np.float32)).astype(ml_dtypes.bfloat16)
    return hi, lo


def kernel(x, Wq, Wk, Wv, Wo):
    global _compiled
    from concourse.bass_utils import run_bass_kernel_spmd

    if _compiled is None:
        _compiled = _build()
    nc = _compiled

    scale = np.float32(1.0 / np.sqrt(O))
    wqh, wql = _split(Wq.astype(np.float32) * scale)
    wkh, wkl = _split(Wk.astype(np.float32))
    wvh = Wv.astype(ml_dtypes.bfloat16)
    woh = Wo.astype(ml_dtypes.bfloat16)

    in_maps = []
    for c in range(NCORES):
        b, half = divmod(c, 2)
        xT = np.ascontiguousarray(x[b].T)          # (E, S) fp32
        xh, xl = _split(xT)
        in_maps.append({
            "xkvh": xh, "xkvl": xl,
            "xqh": np.ascontiguousarray(xh[:, half * SQ:(half + 1) * SQ]),
            "xql": np.ascontiguousarray(xl[:, half * SQ:(half + 1) * SQ]),
            "wqh": wqh, "wql": wql,
            "wkh": wkh, "wkl": wkl, "wvh": wvh, "woh": woh,
        })

    res = run_bass_kernel_spmd(nc, in_maps, core_ids=list(range(NCORES)))

    out = np.empty((B, S, E), dtype=np.float32)
    for c in range(NCORES):
        b, half = divmod(c, 2)
        out[b, half * SQ:(half + 1) * SQ, :] = res.results[c]["out"]
    return out


# revision 17
# speedup vs baseline: 3.3591x; 1.0163x over previous
"""Multi-head self-attention on 8 TRN2 NeuronCores.

Problem: x(4,2048,1024), Wq(8,1024,128), Wk/Wv(1024,128), Wo(1024,1024) fp32.
out = softmax(Q K^T / sqrt(128)) V -> concat heads -> @ Wo.

Sharding: (batch, query-half) across 8 cores — core c handles batch c//2,
query rows [(c%2)*1024, (c%2)*1024+1024). K/V cover the full sequence of the
batch, so each core computes them locally from its x slice; no collectives.

Numerics: scores have std ~1024 and softmax is near-one-hot, so the
x->Q/K->scores chain needs ~fp32 precision. bf16 matmuls with hi/lo split
operands ("split3": Ah*Bh + Ah*Bl + Al*Bh, fp32 PSUM accumulation) give
~5e-6 relative matmul error at 3 cycles/row (native fp32 is 4). The x and
weight splits are precomputed on the host. V/ctx/Wo paths are plain bf16.

Layouts (partition dim first):
  xT (E,S) host-transposed; K^T (O,S) = sum_e Wk[e].T-stationary @ xT[e];
  Q_h^T (O,Sq) likewise (Wq pre-scaled by 1/sqrt(O) on host);
  scores tile (128q, 2048s) = Q^T-slice-stationary @ K^T-moving, fp32 PSUM,
  bank-chunk-major so each 512-col bank finishes early;
  softmax per q-row: per-bank DVE reduce_max -> combine(negate) -> per-bank
  ACT exp(bias=-max, accum_out=den chunk) -> den sum -> 1/den -> DVE scale;
  P transposed 128x128 via PE right after each q-tile (PE gap filler);
  ctx^T (O,Sq) = V-stationary @ P^T-moving; out (Sq,E) = ctx-slices-stationary
  @ Wo-moving (natural output layout).
"""
import numpy as np
import ml_dtypes

B, S, E, H, O = 4, 2048, 1024, 8, 128
SQ = S // 2          # query rows per core
NCORES = 8
ET = E // 128        # 8 e-tiles
ST = S // 128        # 16 s-tiles
QT = SQ // 128       # 8 q-tiles
NB = S // 512        # 4 score banks per q-tile
EC = E // 512        # 2 out-proj column chunks

_compiled = None     # cache so repeated kernel() calls skip rebuild


def _build():
    import concourse.bass as bass
    import concourse.mybir as mybir
    import concourse.tile as tile
    from concourse import bacc
    from concourse.masks import make_identity

    F32 = mybir.dt.float32
    BF16 = mybir.dt.bfloat16
    PS = bass.MemorySpace.PSUM
    EXP = mybir.ActivationFunctionType.Exp

    nc = bacc.Bacc("TRN2", target_bir_lowering=False, debug=False,
                   enable_asserts=True)

    d_xkvh = nc.dram_tensor("xkvh", (E, S), BF16, kind="ExternalInput").ap()
    d_xkvl = nc.dram_tensor("xkvl", (E, S), BF16, kind="ExternalInput").ap()
    d_xqh = nc.dram_tensor("xqh", (E, SQ), BF16, kind="ExternalInput").ap()
    d_xql = nc.dram_tensor("xql", (E, SQ), BF16, kind="ExternalInput").ap()
    d_wqh = nc.dram_tensor("wqh", (H, E, O), BF16, kind="ExternalInput").ap()
    d_wql = nc.dram_tensor("wql", (H, E, O), BF16, kind="ExternalInput").ap()
    d_wkh = nc.dram_tensor("wkh", (E, O), BF16, kind="ExternalInput").ap()
    d_wkl = nc.dram_tensor("wkl", (E, O), BF16, kind="ExternalInput").ap()
    d_wvh = nc.dram_tensor("wvh", (E, O), BF16, kind="ExternalInput").ap()
    d_woh = nc.dram_tensor("woh", (H * O, E), BF16, kind="ExternalInput").ap()
    d_out = nc.dram_tensor("out", (SQ, E), F32, kind="ExternalOutput").ap()

    with tile.TileContext(nc) as tc:
        with (
            tc.tile_pool(name="persist", bufs=1) as persist,
            tc.tile_pool(name="tiny", bufs=24) as tiny,
        ):
            ident = persist.tile([128, 128], BF16, tag="ident")
            make_identity(nc, ident[:])

            wo_sb = persist.tile([128, H, E], BF16, tag="wo")
            nc.sync.dma_start(wo_sb[:], d_woh.rearrange("(h p) e -> p h e", p=128))

            kth = persist.tile([128, S], BF16, tag="kth")
            ktl = persist.tile([128, S], BF16, tag="ktl")
            qth = persist.tile([128, H, SQ], BF16, tag="qth")
            qtl = persist.tile([128, H, SQ], BF16, tag="qtl")
            v_sb = persist.tile([128, ST, O], BF16, tag="v")

            # ---------------- prologue: K^T, V, Q^T projections ----------
            with tc.tile_pool(name="xp", bufs=1) as xp:
                wkh = xp.tile([128, ET, O], BF16, tag="wkh")
                wkl = xp.tile([128, ET, O], BF16, tag="wkl")
                wvh = xp.tile([128, ET, O], BF16, tag="wvh")
                nc.sync.dma_start(wkh[:], d_wkh.rearrange("(t p) o -> p t o", p=128))
                nc.sync.dma_start(wkl[:], d_wkl.rearrange("(t p) o -> p t o", p=128))
                nc.sync.dma_start(wvh[:], d_wvh.rearrange("(t p) o -> p t o", p=128))
                xkvh = xp.tile([128, ET, S], BF16, tag="xkvh")
                xkvl = xp.tile([128, ET, S], BF16, tag="xkvl")
                for e in range(ET):
                    nc.sync.dma_start(xkvh[:, e, :], d_xkvh[e * 128:(e + 1) * 128, :])
                    nc.sync.dma_start(xkvl[:, e, :], d_xkvl[e * 128:(e + 1) * 128, :])

                # K^T (128o x 2048s), split3 accumulation over e
                with tc.tile_pool(name="ktp", bufs=1, space=PS) as ktp:
                    kt_ps = ktp.tile([128, S], F32, tag="kt")
                    for e in range(ET):
                        for ti, (w, xx) in enumerate(
                            ((wkh, xkvh), (wkh, xkvl), (wkl, xkvh))
                        ):
                            for c in range(NB):
                                nc.tensor.matmul(
                                    kt_ps[:, c * 512:(c + 1) * 512],
                                    w[:, e, :],
                                    xx[:, e, c * 512:(c + 1) * 512],
                                    start=(e == 0 and ti == 0),
                                    stop=(e == ET - 1 and ti == 2),
                                )
                    nc.scalar.copy(kth[:], kt_ps[:])
                    nc.vector.tensor_sub(ktl[:], kt_ps[:], kth[:])

                with (
                    tc.tile_pool(name="vp", bufs=2, space=PS) as vp,
                    tc.tile_pool(name="qp", bufs=2, space=PS) as qp,
                ):
                    # V (s-part tiles), plain bf16
                    for st in range(ST):
                        v_ps = vp.tile([128, O], F32, tag="vps")
                        for e in range(ET):
                            nc.tensor.matmul(
                                v_ps[:],
                                xkvh[:, e, st * 128:(st + 1) * 128],
                                wvh[:, e, :],
                                start=(e == 0),
                                stop=(e == ET - 1),
                            )
                        nc.vector.tensor_copy(v_sb[:, st, :], v_ps[:])

                    # Q^T per head (Wq pre-scaled by 1/sqrt(O) on host)
                    xqh = xp.tile([128, ET, SQ], BF16, tag="xqh")
                    xql = xp.tile([128, ET, SQ], BF16, tag="xql")
                    for e in range(ET):
                        nc.sync.dma_start(xqh[:, e, :], d_xqh[e * 128:(e + 1) * 128, :])
                        nc.sync.dma_start(xql[:, e, :], d_xql[e * 128:(e + 1) * 128, :])
                    wqh = xp.tile([128, H, ET, O], BF16, tag="wqh")
                    wql = xp.tile([128, H, ET, O], BF16, tag="wql")
                    nc.sync.dma_start(
                        wqh[:], d_wqh.rearrange("h (t p) o -> p h t o", p=128))
                    nc.sync.dma_start(
                        wql[:], d_wql.rearrange("h (t p) o -> p h t o", p=128))

                    for h in range(H):
                        q_ps = qp.tile([128, SQ], F32, tag="qtps")
                        for e in range(ET):
                            for ti, (w, xx) in enumerate(
                                ((wqh, xqh), (wqh, xql), (wql, xqh))
                            ):
                                for c in range(SQ // 512):
                                    nc.tensor.matmul(
                                        q_ps[:, c * 512:(c + 1) * 512],
                                        w[:, h, e, :],
                                        xx[:, e, c * 512:(c + 1) * 512],
                                        start=(e == 0 and ti == 0),
                                        stop=(e == ET - 1 and ti == 2),
                                    )
                        nc.scalar.copy(qth[:, h, :], q_ps[:])
                        nc.vector.tensor_sub(qtl[:, h, :], q_ps[:], qth[:, h, :])

            # ---------------- main: per-head attention ------------------
            # PSUM budget (8 banks): "acc1024" 2-bank tiles x3 bufs shared by
            # score-halves, ctx and out accumulators (6 banks) + one 2-bank
            # transpose staging tile.  Score halves cycle through 3 slots so
            # the next q-tile's matmuls never wait on this one's softmax.
            with (
                tc.tile_pool(name="p_pool", bufs=4) as p_pool,
                tc.tile_pool(name="pt_pool", bufs=2) as pt_pool,
                tc.tile_pool(name="ctx_pool", bufs=H) as ctx_pool,
                tc.tile_pool(name="acc_ps", bufs=3, space=PS) as acc_psp,
                tc.tile_pool(name="pt_ps", bufs=1, space=PS) as pt_psp,
                tc.tile_pool(name="o_sb", bufs=2) as o_sbp,
            ):
                HS = S // 2  # 1024-wide score half

                def emit_transposes(pt_h, p_qt, qt):
                    # runs one q-tile BEHIND the softmax pipeline: all deps
                    # are long resolved, so these are always-ready PE filler
                    # and the ACT copy never stalls the exp stream
                    pt_ps = pt_psp.tile([128, ST, 128], BF16, tag="ptps")
                    for st in range(ST):
                        nc.tensor.transpose(
                            pt_ps[:, st, :],
                            p_qt[:, st * 128:(st + 1) * 128],
                            ident[:],
                        )
                    nc.scalar.copy(
                        pt_h[:, :, qt * 128:(qt + 1) * 128], pt_ps[:])

                ctxs = []
                for h in range(H):
                    pt_h = pt_pool.tile([128, ST, SQ], BF16, tag="pt")
                    lagged = None
                    for qt in range(QT):
                        negmax4 = tiny.tile([128, NB], F32, tag="negmax4")
                        den2 = tiny.tile([128, 2], F32, tag="den2")
                        halves = []
                        for sh in range(2):
                            s_ps = acc_psp.tile([128, HS], F32, tag="acc1024")
                            halves.append(s_ps)
                            for ti, (qq, kk) in enumerate(
                                ((qth, kth), (qth, ktl), (qtl, kth))
                            ):
                                for c in range(2):
                                    nc.tensor.matmul(
                                        s_ps[:, c * 512:(c + 1) * 512],
                                        qq[:, h, qt * 128:(qt + 1) * 128],
                                        kk[:, sh * HS + c * 512:
                                           sh * HS + (c + 1) * 512],
                                        start=(ti == 0),
                                        stop=(ti == 2),
                                    )
                            for c in range(2):
                                nc.vector.reduce_max(
                                    out=negmax4[:, sh * 2 + c:sh * 2 + c + 1],
                                    in_=s_ps[:, c * 512:(c + 1) * 512],
                                    axis=mybir.AxisListType.X,
                                )
                        negmax = tiny.tile([128, 1], F32, tag="negmax")
                        nc.vector.reduce_max(
                            out=negmax[:], in_=negmax4[:],
                            axis=mybir.AxisListType.X, negate=True,
                        )
                        p_qt = p_pool.tile([128, S], BF16, tag="p")
                        for sh in range(2):
                            nc.scalar.activation(
                                p_qt[:, sh * HS:(sh + 1) * HS],
                                halves[sh][:],
                                EXP, bias=negmax[:], scale=1.0,
                                accum_out=den2[:, sh:sh + 1],
                            )
                        den = tiny.tile([128, 1], F32, tag="den")
                        nc.vector.reduce_sum(
                            out=den[:], in_=den2[:], axis=mybir.AxisListType.X)
                        invden = tiny.tile([128, 1], F32, tag="invden")
                        nc.vector.reciprocal(invden[:], den[:])
                        nc.vector.tensor_scalar_mul(p_qt[:], p_qt[:], invden[:])

                        if lagged is not None:
                            emit_transposes(pt_h, *lagged)
                        lagged = (p_qt, qt)
                    emit_transposes(pt_h, *lagged)

                    # ctx^T (o-part, q-free) accumulated over s-tiles
                    ct_ps = acc_psp.tile([128, SQ], F32, tag="acc1024")
                    for qc in range(SQ // 512):
                        for st in range(ST):
                            nc.tensor.matmul(
                                ct_ps[:, qc * 512:(qc + 1) * 512],
                                v_sb[:, st, :],
                                pt_h[:, st, qc * 512:(qc + 1) * 512],
                                start=(st == 0),
                                stop=(st == ST - 1),
                            )
                    ctx_h = ctx_pool.tile([128, SQ], BF16, tag="ctx")
                    nc.vector.tensor_copy(ctx_h[:], ct_ps[:])
                    ctxs.append(ctx_h)

                # ------- out (q-part, e-free) = sum_h ctx_h^T-slices @ Wo_h
                for qt in range(QT):
                    o_ps = acc_psp.tile([128, E], F32, tag="acc1024")
                    for h in range(H):
                        for ec in range(EC):
                            nc.tensor.matmul(
                                o_ps[:, ec * 512:(ec + 1) * 512],
                                ctxs[h][:, qt * 128:(qt + 1) * 128],
                                wo_sb[:, h, ec * 512:(ec + 1) * 512],
                                start=(h == 0),
                                stop=(h == H - 1),
                            )
                    o_sb = o_sbp.tile([128, E], F32, tag="osb")
                    nc.vector.tensor_copy(o_sb[:], o_ps[:])
                    nc.sync.dma_start(d_out[qt * 128:(qt + 1) * 128, :], o_sb[:])

    nc.compile()
    return nc


def _split(a):
    """fp32 -> (hi, lo) bf16 pair with hi + lo ~= a."""
    hi = a.astype(ml_dtypes.bfloat16)
    lo = (a - hi.astype(np.float32)).astype(ml_dtypes.bfloat16)
    return hi, lo


def kernel(x, Wq, Wk, Wv, Wo):
    global _compiled
    from concourse.bass_utils import run_bass_kernel_spmd

    if _compiled is None:
        _compiled = _build()
    nc = _compiled

    scale = np.float32(1.0 / np.sqrt(O))
    wqh, wql = _split(Wq.astype(np.float32) * scale)
    wkh, wkl = _split(Wk.astype(np.float32))
    wvh = Wv.astype(ml_dtypes.bfloat16)
    woh = Wo.astype(ml_dtypes.bfloat16)

    in_maps = []
    for c in range(NCORES):
        b, half = divmod(c, 2)
        xT = np.ascontiguousarray(x[b].T)          # (E, S) fp32
        xh, xl = _split(xT)
        in_maps.append({
            "xkvh": xh, "xkvl": xl,
            "xqh": np.ascontiguousarray(xh[:, half * SQ:(half + 1) * SQ]),
            "xql": np.ascontiguousarray(xl[:, half * SQ:(half + 1) * SQ]),
            "wqh": wqh, "wql": wql,
            "wkh": wkh, "wkl": wkl, "wvh": wvh, "woh": woh,
        })

    res = run_bass_kernel_spmd(nc, in_maps, core_ids=list(range(NCORES)))

    out = np.empty((B, S, E), dtype=np.float32)
    for c in range(NCORES):
        b, half = divmod(c, 2)
        out[b, half * SQ:(half + 1) * SQ, :] = res.results[c]["out"]
    return out


# revision 18
# speedup vs baseline: 3.6425x; 1.0844x over previous
"""Multi-head self-attention on 8 TRN2 NeuronCores.

Problem: x(4,2048,1024), Wq(8,1024,128), Wk/Wv(1024,128), Wo(1024,1024) fp32.
out = softmax(Q K^T / sqrt(128)) V -> concat heads -> @ Wo.

Sharding: (batch, query-half) across 8 cores — core c handles batch c//2,
query rows [(c%2)*1024, (c%2)*1024+1024). K/V cover the full sequence of the
batch, so each core computes them locally from its x slice; no collectives.

Numerics: scores have std ~1024 and softmax is near-one-hot, so the
x->Q/K->scores chain needs ~fp32 precision. bf16 matmuls with hi/lo split
operands ("split3": Ah*Bh + Ah*Bl + Al*Bh, fp32 PSUM accumulation) give
~5e-6 relative matmul error at 3 cycles/row (native fp32 is 4). The x and
weight splits are precomputed on the host. V/ctx/Wo paths are plain bf16.

Layouts (partition dim first):
  xT (E,S) host-transposed; K^T (O,S) = sum_e Wk[e].T-stationary @ xT[e];
  Q_h^T (O,Sq) likewise (Wq pre-scaled by 1/sqrt(O) on host);
  scores tile (128q, 2048s) = Q^T-slice-stationary @ K^T-moving, fp32 PSUM,
  bank-chunk-major so each 512-col bank finishes early;
  softmax per q-row: per-bank DVE reduce_max -> combine(negate) -> per-bank
  ACT exp(bias=-max, accum_out=den chunk) -> den sum -> 1/den -> DVE scale;
  P transposed 128x128 via PE right after each q-tile (PE gap filler);
  ctx^T (O,Sq) = V-stationary @ P^T-moving; out (Sq,E) = ctx-slices-stationary
  @ Wo-moving (natural output layout).
"""
import numpy as np
import ml_dtypes

B, S, E, H, O = 4, 2048, 1024, 8, 128
SQ = S // 2          # query rows per core
NCORES = 8
ET = E // 128        # 8 e-tiles
ST = S // 128        # 16 s-tiles
QT = SQ // 128       # 8 q-tiles
NB = S // 512        # 4 score banks per q-tile
EC = E // 512        # 2 out-proj column chunks

_compiled = None     # cache so repeated kernel() calls skip rebuild


def _build():
    import concourse.bass as bass
    import concourse.mybir as mybir
    import concourse.tile as tile
    from concourse import bacc
    from concourse.masks import make_identity

    F32 = mybir.dt.float32
    BF16 = mybir.dt.bfloat16
    PS = bass.MemorySpace.PSUM
    EXP = mybir.ActivationFunctionType.Exp

    nc = bacc.Bacc("TRN2", target_bir_lowering=False, debug=False,
                   enable_asserts=True)

    d_xkvh = nc.dram_tensor("xkvh", (E, S), BF16, kind="ExternalInput").ap()
    d_xkvl = nc.dram_tensor("xkvl", (E, S), BF16, kind="ExternalInput").ap()
    d_xqh = nc.dram_tensor("xqh", (E, SQ), BF16, kind="ExternalInput").ap()
    d_xql = nc.dram_tensor("xql", (E, SQ), BF16, kind="ExternalInput").ap()
    d_wqh = nc.dram_tensor("wqh", (H, E, O), BF16, kind="ExternalInput").ap()
    d_wql = nc.dram_tensor("wql", (H, E, O), BF16, kind="ExternalInput").ap()
    d_wkh = nc.dram_tensor("wkh", (E, O), BF16, kind="ExternalInput").ap()
    d_wkl = nc.dram_tensor("wkl", (E, O), BF16, kind="ExternalInput").ap()
    d_wvh = nc.dram_tensor("wvh", (E, O), BF16, kind="ExternalInput").ap()
    d_woh = nc.dram_tensor("woh", (H * O, E), BF16, kind="ExternalInput").ap()
    d_out = nc.dram_tensor("out", (SQ, E), F32, kind="ExternalOutput").ap()

    with tile.TileContext(nc) as tc:
        with (
            tc.tile_pool(name="persist", bufs=1) as persist,
            tc.tile_pool(name="tiny", bufs=24) as tiny,
        ):
            ident = persist.tile([128, 128], BF16, tag="ident")
            make_identity(nc, ident[:])

            wo_sb = persist.tile([128, H, E], BF16, tag="wo")
            nc.sync.dma_start(wo_sb[:], d_woh.rearrange("(h p) e -> p h e", p=128))

            kth = persist.tile([128, S], BF16, tag="kth")
            ktl = persist.tile([128, S], BF16, tag="ktl")
            qth = persist.tile([128, H, SQ], BF16, tag="qth")
            qtl = persist.tile([128, H, SQ], BF16, tag="qtl")
            v_sb = persist.tile([128, ST, O], BF16, tag="v")

            # ---------------- prologue: K^T, V, Q^T projections ----------
            with tc.tile_pool(name="xp", bufs=1) as xp:
                wkh = xp.tile([128, ET, O], BF16, tag="wkh")
                wkl = xp.tile([128, ET, O], BF16, tag="wkl")
                wvh = xp.tile([128, ET, O], BF16, tag="wvh")
                nc.sync.dma_start(wkh[:], d_wkh.rearrange("(t p) o -> p t o", p=128))
                nc.sync.dma_start(wkl[:], d_wkl.rearrange("(t p) o -> p t o", p=128))
                nc.sync.dma_start(wvh[:], d_wvh.rearrange("(t p) o -> p t o", p=128))
                xkvh = xp.tile([128, ET, S], BF16, tag="xkvh")
                xkvl = xp.tile([128, ET, S], BF16, tag="xkvl")
                for e in range(ET):
                    nc.sync.dma_start(xkvh[:, e, :], d_xkvh[e * 128:(e + 1) * 128, :])
                    nc.sync.dma_start(xkvl[:, e, :], d_xkvl[e * 128:(e + 1) * 128, :])

                # K^T (128o x 2048s), split3 accumulation over e
                with tc.tile_pool(name="ktp", bufs=1, space=PS) as ktp:
                    kt_ps = ktp.tile([128, S], F32, tag="kt")
                    for e in range(ET):
                        for ti, (w, xx) in enumerate(
                            ((wkh, xkvh), (wkh, xkvl), (wkl, xkvh))
                        ):
                            for c in range(NB):
                                nc.tensor.matmul(
                                    kt_ps[:, c * 512:(c + 1) * 512],
                                    w[:, e, :],
                                    xx[:, e, c * 512:(c + 1) * 512],
                                    start=(e == 0 and ti == 0),
                                    stop=(e == ET - 1 and ti == 2),
                                )
                    nc.scalar.copy(kth[:], kt_ps[:])
                    nc.vector.tensor_sub(ktl[:], kt_ps[:], kth[:])

                with (
                    tc.tile_pool(name="vp", bufs=2, space=PS) as vp,
                    tc.tile_pool(name="qp", bufs=2, space=PS) as qp,
                ):
                    # V (s-part tiles), plain bf16
                    for st in range(ST):
                        v_ps = vp.tile([128, O], F32, tag="vps")
                        for e in range(ET):
                            nc.tensor.matmul(
                                v_ps[:],
                                xkvh[:, e, st * 128:(st + 1) * 128],
                                wvh[:, e, :],
                                start=(e == 0),
                                stop=(e == ET - 1),
                            )
                        nc.vector.tensor_copy(v_sb[:, st, :], v_ps[:])

                    # Q^T per head (Wq pre-scaled by 1/sqrt(O) on host)
                    xqh = xp.tile([128, ET, SQ], BF16, tag="xqh")
                    xql = xp.tile([128, ET, SQ], BF16, tag="xql")
                    for e in range(ET):
                        nc.sync.dma_start(xqh[:, e, :], d_xqh[e * 128:(e + 1) * 128, :])
                        nc.sync.dma_start(xql[:, e, :], d_xql[e * 128:(e + 1) * 128, :])
                    wqh = xp.tile([128, H, ET, O], BF16, tag="wqh")
                    wql = xp.tile([128, H, ET, O], BF16, tag="wql")
                    nc.sync.dma_start(
                        wqh[:], d_wqh.rearrange("h (t p) o -> p h t o", p=128))
                    nc.sync.dma_start(
                        wql[:], d_wql.rearrange("h (t p) o -> p h t o", p=128))

                    for h in range(H):
                        q_ps = qp.tile([128, SQ], F32, tag="qtps")
                        for e in range(ET):
                            for ti, (w, xx) in enumerate(
                                ((wqh, xqh), (wqh, xql), (wql, xqh))
                            ):
                                for c in range(SQ // 512):
                                    nc.tensor.matmul(
                                        q_ps[:, c * 512:(c + 1) * 512],
                                        w[:, h, e, :],
                                        xx[:, e, c * 512:(c + 1) * 512],
                                        start=(e == 0 and ti == 0),
                                        stop=(e == ET - 1 and ti == 2),
                                    )
                        nc.scalar.copy(qth[:, h, :], q_ps[:])
                        nc.vector.tensor_sub(qtl[:, h, :], q_ps[:], qth[:, h, :])

            # ---------------- main: per-head attention ------------------
            # PSUM budget (8 banks): "acc1024" 2-bank tiles x3 bufs shared by
            # score-halves, ctx and out accumulators (6 banks) + one 2-bank
            # transpose staging tile.  Score halves cycle through 3 slots so
            # the next q-tile's matmuls never wait on this one's softmax.
            with (
                tc.tile_pool(name="p_pool", bufs=4) as p_pool,
                tc.tile_pool(name="pt_pool", bufs=2) as pt_pool,
                tc.tile_pool(name="ctx_pool", bufs=H) as ctx_pool,
                tc.tile_pool(name="acc_ps", bufs=3, space=PS) as acc_psp,
                tc.tile_pool(name="pt_ps", bufs=1, space=PS) as pt_psp,
                tc.tile_pool(name="o_sb", bufs=2) as o_sbp,
            ):
                HS = S // 2  # 1024-wide score half

                def emit_transposes(pt_h, p_qt, qt):
                    # runs one q-tile BEHIND the softmax pipeline: all deps
                    # are long resolved, so these are always-ready PE filler
                    # and the ACT copy never stalls the exp stream
                    pt_ps = pt_psp.tile([128, ST, 128], BF16, tag="ptps")
                    for st in range(ST):
                        nc.tensor.transpose(
                            pt_ps[:, st, :],
                            p_qt[:, st * 128:(st + 1) * 128],
                            ident[:],
                        )
                    cp = nc.scalar.copy if qt % 2 else nc.vector.tensor_copy
                    cp(pt_h[:, :, qt * 128:(qt + 1) * 128], pt_ps[:])

                ctxs = []
                for h in range(H):
                    pt_h = pt_pool.tile([128, ST, SQ], BF16, tag="pt")
                    lagged = None
                    for qt in range(QT):
                        negmax4 = tiny.tile([128, NB], F32, tag="negmax4")
                        den2 = tiny.tile([128, 2], F32, tag="den2")
                        halves = []
                        for sh in range(2):
                            s_ps = acc_psp.tile([128, HS], F32, tag="acc1024")
                            halves.append(s_ps)
                            for ti, (qq, kk) in enumerate(
                                ((qth, kth), (qth, ktl), (qtl, kth))
                            ):
                                for c in range(2):
                                    nc.tensor.matmul(
                                        s_ps[:, c * 512:(c + 1) * 512],
                                        qq[:, h, qt * 128:(qt + 1) * 128],
                                        kk[:, sh * HS + c * 512:
                                           sh * HS + (c + 1) * 512],
                                        start=(ti == 0),
                                        stop=(ti == 2),
                                    )
                            for c in range(2):
                                nc.vector.reduce_max(
                                    out=negmax4[:, sh * 2 + c:sh * 2 + c + 1],
                                    in_=s_ps[:, c * 512:(c + 1) * 512],
                                    axis=mybir.AxisListType.X,
                                )
                        negmax = tiny.tile([128, 1], F32, tag="negmax")
                        nc.vector.reduce_max(
                            out=negmax[:], in_=negmax4[:],
                            axis=mybir.AxisListType.X, negate=True,
                        )
                        p_qt = p_pool.tile([128, S], BF16, tag="p")
                        for sh in range(2):
                            nc.scalar.activation(
                                p_qt[:, sh * HS:(sh + 1) * HS],
                                halves[sh][:],
                                EXP, bias=negmax[:], scale=1.0,
                                accum_out=den2[:, sh:sh + 1],
                            )
                        den = tiny.tile([128, 1], F32, tag="den")
                        nc.vector.reduce_sum(
                            out=den[:], in_=den2[:], axis=mybir.AxisListType.X)
                        invden = tiny.tile([128, 1], F32, tag="invden")
                        nc.vector.reciprocal(invden[:], den[:])
                        nc.vector.tensor_scalar_mul(p_qt[:], p_qt[:], invden[:])

                        if lagged is not None:
                            emit_transposes(pt_h, *lagged)
                        lagged = (p_qt, qt)
                    emit_transposes(pt_h, *lagged)

                    # ctx^T (o-part, q-free) accumulated over s-tiles
                    ct_ps = acc_psp.tile([128, SQ], F32, tag="acc1024")
                    for qc in range(SQ // 512):
                        for st in range(ST):
                            nc.tensor.matmul(
                                ct_ps[:, qc * 512:(qc + 1) * 512],
                                v_sb[:, st, :],
                                pt_h[:, st, qc * 512:(qc + 1) * 512],
                                start=(st == 0),
                                stop=(st == ST - 1),
                            )
                    ctx_h = ctx_pool.tile([128, SQ], BF16, tag="ctx")
                    nc.vector.tensor_copy(ctx_h[:], ct_ps[:])
                    ctxs.append(ctx_h)

                # ------- out (q-part, e-free) = sum_h ctx_h^T-slices @ Wo_h
                for qt in range(QT):
                    o_ps = acc_psp.tile([128, E], F32, tag="acc1024")
                    for h in range(H):
                        for ec in range(EC):
                            nc.tensor.matmul(
                                o_ps[:, ec * 512:(ec + 1) * 512],
                                ctxs[h][:, qt * 128:(qt + 1) * 128],
                                wo_sb[:, h, ec * 512:(ec + 1) * 512],
                                start=(h == 0),
                                stop=(h == H - 1),
                            )
                    o_sb = o_sbp.tile([128, E], F32, tag="osb")
                    nc.vector.tensor_copy(o_sb[:], o_ps[:])
                    nc.sync.dma_start(d_out[qt * 128:(qt + 1) * 128, :], o_sb[:])

    nc.compile()
    return nc


def _split(a):
    """fp32 -> (hi, lo) bf16 pair with hi + lo ~= a."""
    hi = a.astype(ml_dtypes.bfloat16)
    lo = (a - hi.astype(np.float32)).astype(ml_dtypes.bfloat16)
    return hi, lo


def kernel(x, Wq, Wk, Wv, Wo):
    global _compiled
    from concourse.bass_utils import run_bass_kernel_spmd

    if _compiled is None:
        _compiled = _build()
    nc = _compiled

    scale = np.float32(1.0 / np.sqrt(O))
    wqh, wql = _split(Wq.astype(np.float32) * scale)
    wkh, wkl = _split(Wk.astype(np.float32))
    wvh = Wv.astype(ml_dtypes.bfloat16)
    woh = Wo.astype(ml_dtypes.bfloat16)

    in_maps = []
    for c in range(NCORES):
        b, half = divmod(c, 2)
        xT = np.ascontiguousarray(x[b].T)          # (E, S) fp32
        xh, xl = _split(xT)
        in_maps.append({
            "xkvh": xh, "xkvl": xl,
            "xqh": np.ascontiguousarray(xh[:, half * SQ:(half + 1) * SQ]),
            "xql": np.ascontiguousarray(xl[:, half * SQ:(half + 1) * SQ]),
            "wqh": wqh, "wql": wql,
            "wkh": wkh, "wkl": wkl, "wvh": wvh, "woh": woh,
        })

    res = run_bass_kernel_spmd(nc, in_maps, core_ids=list(range(NCORES)))

    out = np.empty((B, S, E), dtype=np.float32)
    for c in range(NCORES):
        b, half = divmod(c, 2)
        out[b, half * SQ:(half + 1) * SQ, :] = res.results[c]["out"]
    return out


# revision 20
# speedup vs baseline: 4.4829x; 1.2307x over previous
"""Multi-head self-attention on 8 TRN2 NeuronCores.

Problem: x(4,2048,1024), Wq(8,1024,128), Wk/Wv(1024,128), Wo(1024,1024) fp32.
out = softmax(Q K^T / sqrt(128)) V -> concat heads -> @ Wo.

Sharding: (batch, query-half) across 8 cores — core c handles batch c//2,
query rows [(c%2)*1024, (c%2)*1024+1024). K/V cover the full sequence of the
batch, so each core computes them locally from its x slice; no collectives.

Numerics: scores have std ~1024 and softmax is near-one-hot, so the
x->Q/K->scores chain needs ~fp32 precision. bf16 matmuls with hi/lo split
operands ("split3": Ah*Bh + Ah*Bl + Al*Bh, fp32 PSUM accumulation) give
~5e-6 relative matmul error at 3 cycles/row (native fp32 is 4). The x and
weight splits are precomputed on the host. V/ctx/Wo paths are plain bf16.

Layouts (partition dim first):
  xT (E,S) host-transposed; K^T (O,S) = sum_e Wk[e].T-stationary @ xT[e];
  Q_h^T (O,Sq) likewise (Wq pre-scaled by 1/sqrt(O) on host);
  scores tile (128q, 2048s) = Q^T-slice-stationary @ K^T-moving, fp32 PSUM,
  bank-chunk-major so each 512-col bank finishes early;
  softmax per q-row: per-bank DVE reduce_max -> combine(negate) -> per-bank
  ACT exp(bias=-max, accum_out=den chunk) -> den sum -> 1/den -> DVE scale;
  P transposed 128x128 via PE right after each q-tile (PE gap filler);
  ctx^T (O,Sq) = V-stationary @ P^T-moving; out (Sq,E) = ctx-slices-stationary
  @ Wo-moving (natural output layout).
"""
import numpy as np
import ml_dtypes

B, S, E, H, O = 4, 2048, 1024, 8, 128
SQ = S // 2          # query rows per core
NCORES = 8
ET = E // 128        # 8 e-tiles
ST = S // 128        # 16 s-tiles
QT = SQ // 128       # 8 q-tiles
NB = S // 512        # 4 score banks per q-tile
EC = E // 512        # 2 out-proj column chunks

_compiled = None     # cache so repeated kernel() calls skip rebuild


def _build():
    import concourse.bass as bass
    import concourse.mybir as mybir
    import concourse.tile as tile
    from concourse import bacc
    from concourse.masks import make_identity

    F32 = mybir.dt.float32
    BF16 = mybir.dt.bfloat16
    PS = bass.MemorySpace.PSUM
    EXP = mybir.ActivationFunctionType.Exp

    nc = bacc.Bacc("TRN2", target_bir_lowering=False, debug=False,
                   enable_asserts=True)

    d_xkvh = nc.dram_tensor("xkvh", (E, S), BF16, kind="ExternalInput").ap()
    d_xkvl = nc.dram_tensor("xkvl", (E, S), BF16, kind="ExternalInput").ap()
    d_xqh = nc.dram_tensor("xqh", (E, SQ), BF16, kind="ExternalInput").ap()
    d_xql = nc.dram_tensor("xql", (E, SQ), BF16, kind="ExternalInput").ap()
    d_wqh = nc.dram_tensor("wqh", (H, E, O), BF16, kind="ExternalInput").ap()
    d_wql = nc.dram_tensor("wql", (H, E, O), BF16, kind="ExternalInput").ap()
    d_wkh = nc.dram_tensor("wkh", (E, O), BF16, kind="ExternalInput").ap()
    d_wkl = nc.dram_tensor("wkl", (E, O), BF16, kind="ExternalInput").ap()
    d_wvh = nc.dram_tensor("wvh", (E, O), BF16, kind="ExternalInput").ap()
    d_woh = nc.dram_tensor("woh", (H * O, E), BF16, kind="ExternalInput").ap()
    d_out = nc.dram_tensor("out", (SQ, E), F32, kind="ExternalOutput").ap()

    with tile.TileContext(nc) as tc:
        with (
            tc.tile_pool(name="persist", bufs=1) as persist,
            tc.tile_pool(name="tiny", bufs=24) as tiny,
        ):
            ident = persist.tile([128, 128], BF16, tag="ident")
            make_identity(nc, ident[:])

            wo_sb = persist.tile([128, H, E], BF16, tag="wo")
            nc.sync.dma_start(wo_sb[:], d_woh.rearrange("(h p) e -> p h e", p=128))

            kth = persist.tile([128, S], BF16, tag="kth")
            ktl = persist.tile([128, S], BF16, tag="ktl")
            qth = persist.tile([128, H, SQ], BF16, tag="qth")
            qtl = persist.tile([128, H, SQ], BF16, tag="qtl")
            v_sb = persist.tile([128, ST, O], BF16, tag="v")

            # ---------------- prologue: K^T, V, Q^T projections ----------
            with tc.tile_pool(name="xp", bufs=1) as xp:
                wkh = xp.tile([128, ET, O], BF16, tag="wkh")
                wkl = xp.tile([128, ET, O], BF16, tag="wkl")
                wvh = xp.tile([128, ET, O], BF16, tag="wvh")
                nc.sync.dma_start(wkh[:], d_wkh.rearrange("(t p) o -> p t o", p=128))
                nc.sync.dma_start(wkl[:], d_wkl.rearrange("(t p) o -> p t o", p=128))
                nc.sync.dma_start(wvh[:], d_wvh.rearrange("(t p) o -> p t o", p=128))
                xkvh = xp.tile([128, ET, S], BF16, tag="xkvh")
                xkvl = xp.tile([128, ET, S], BF16, tag="xkvl")
                for e in range(ET):
                    nc.sync.dma_start(xkvh[:, e, :], d_xkvh[e * 128:(e + 1) * 128, :])
                    nc.sync.dma_start(xkvl[:, e, :], d_xkvl[e * 128:(e + 1) * 128, :])

                # K^T (128o x 2048s), split3 accumulation over e
                with tc.tile_pool(name="ktp", bufs=1, space=PS) as ktp:
                    kt_ps = ktp.tile([128, S], F32, tag="kt")
                    for e in range(ET):
                        for ti, (w, xx) in enumerate(
                            ((wkh, xkvh), (wkh, xkvl), (wkl, xkvh))
                        ):
                            for c in range(NB):
                                nc.tensor.matmul(
                                    kt_ps[:, c * 512:(c + 1) * 512],
                                    w[:, e, :],
                                    xx[:, e, c * 512:(c + 1) * 512],
                                    start=(e == 0 and ti == 0),
                                    stop=(e == ET - 1 and ti == 2),
                                )
                    nc.scalar.copy(kth[:], kt_ps[:])
                    nc.vector.tensor_sub(ktl[:], kt_ps[:], kth[:])

                with (
                    tc.tile_pool(name="vp", bufs=2, space=PS) as vp,
                    tc.tile_pool(name="qp", bufs=2, space=PS) as qp,
                ):
                    # V (s-part tiles), plain bf16
                    for st in range(ST):
                        v_ps = vp.tile([128, O], F32, tag="vps")
                        for e in range(ET):
                            nc.tensor.matmul(
                                v_ps[:],
                                xkvh[:, e, st * 128:(st + 1) * 128],
                                wvh[:, e, :],
                                start=(e == 0),
                                stop=(e == ET - 1),
                            )
                        nc.vector.tensor_copy(v_sb[:, st, :], v_ps[:])

                    # Q^T per head (Wq pre-scaled by 1/sqrt(O) on host)
                    xqh = xp.tile([128, ET, SQ], BF16, tag="xqh")
                    xql = xp.tile([128, ET, SQ], BF16, tag="xql")
                    for e in range(ET):
                        nc.sync.dma_start(xqh[:, e, :], d_xqh[e * 128:(e + 1) * 128, :])
                        nc.sync.dma_start(xql[:, e, :], d_xql[e * 128:(e + 1) * 128, :])
                    wqh = xp.tile([128, H, ET, O], BF16, tag="wqh")
                    wql = xp.tile([128, H, ET, O], BF16, tag="wql")
                    nc.sync.dma_start(
                        wqh[:], d_wqh.rearrange("h (t p) o -> p h t o", p=128))
                    nc.sync.dma_start(
                        wql[:], d_wql.rearrange("h (t p) o -> p h t o", p=128))

                    for h in range(H):
                        q_ps = qp.tile([128, SQ], F32, tag="qtps")
                        for e in range(ET):
                            for ti, (w, xx) in enumerate(
                                ((wqh, xqh), (wqh, xql), (wql, xqh))
                            ):
                                for c in range(SQ // 512):
                                    nc.tensor.matmul(
                                        q_ps[:, c * 512:(c + 1) * 512],
                                        w[:, h, e, :],
                                        xx[:, e, c * 512:(c + 1) * 512],
                                        start=(e == 0 and ti == 0),
                                        stop=(e == ET - 1 and ti == 2),
                                    )
                        nc.scalar.copy(qth[:, h, :], q_ps[:])
                        nc.vector.tensor_sub(qtl[:, h, :], q_ps[:], qth[:, h, :])

            # ---------------- main: per-head attention ------------------
            # PSUM budget (8 banks): "acc1024" 2-bank tiles x3 bufs shared by
            # score-halves, ctx and out accumulators (6 banks) + one 2-bank
            # transpose staging tile.  Score halves cycle through 3 slots so
            # the next q-tile's matmuls never wait on this one's softmax.
            with (
                tc.tile_pool(name="p_pool", bufs=4) as p_pool,
                tc.tile_pool(name="pt_pool", bufs=2) as pt_pool,
                tc.tile_pool(name="ctx_pool", bufs=H) as ctx_pool,
                tc.tile_pool(name="acc_ps", bufs=3, space=PS) as acc_psp,
                tc.tile_pool(name="pt_ps", bufs=1, space=PS) as pt_psp,
                tc.tile_pool(name="o_sb", bufs=2) as o_sbp,
            ):
                HS = S // 2  # 1024-wide score half

                def emit_transposes(pt_h, p_qt, qt):
                    # runs one q-tile BEHIND the softmax pipeline: all deps
                    # are long resolved, so these are always-ready PE filler
                    # and the ACT copy never stalls the exp stream
                    pt_ps = pt_psp.tile([128, ST, 128], BF16, tag="ptps")
                    for st in range(ST):
                        nc.tensor.transpose(
                            pt_ps[:, st, :],
                            p_qt[:, st * 128:(st + 1) * 128],
                            ident[:],
                        )
                    cp = nc.scalar.copy if qt % 2 else nc.vector.tensor_copy
                    cp(pt_h[:, :, qt * 128:(qt + 1) * 128], pt_ps[:])

                MIN = mybir.AluOpType.min
                SUB = mybir.AluOpType.subtract
                ctxs = []
                for h in range(H):
                    pt_h = pt_pool.tile([128, ST, SQ], BF16, tag="pt")
                    lagged = None
                    for qt in range(QT):
                        # flash-style: each half gets a LOCAL max + exp so its
                        # PSUM slot frees without waiting for the other half;
                        # tiny per-partition factors fix up the normalization.
                        negmax4 = tiny.tile([128, NB], F32, tag="negmax4")
                        nm2 = tiny.tile([128, 2], F32, tag="nm2")
                        den2 = tiny.tile([128, 2], F32, tag="den2")
                        p_qt = p_pool.tile([128, S], BF16, tag="p")
                        for sh in range(2):
                            s_ps = acc_psp.tile([128, HS], F32, tag="acc1024")
                            for ti, (qq, kk) in enumerate(
                                ((qth, kth), (qth, ktl), (qtl, kth))
                            ):
                                for c in range(2):
                                    nc.tensor.matmul(
                                        s_ps[:, c * 512:(c + 1) * 512],
                                        qq[:, h, qt * 128:(qt + 1) * 128],
                                        kk[:, sh * HS + c * 512:
                                           sh * HS + (c + 1) * 512],
                                        start=(ti == 0),
                                        stop=(ti == 2),
                                    )
                            for c in range(2):
                                nc.vector.reduce_max(
                                    out=negmax4[:, sh * 2 + c:sh * 2 + c + 1],
                                    in_=s_ps[:, c * 512:(c + 1) * 512],
                                    axis=mybir.AxisListType.X,
                                )
                            nc.vector.reduce_max(
                                out=nm2[:, sh:sh + 1],
                                in_=negmax4[:, sh * 2:sh * 2 + 2],
                                axis=mybir.AxisListType.X, negate=True,
                            )
                            nc.scalar.activation(
                                p_qt[:, sh * HS:(sh + 1) * HS],
                                s_ps[:],
                                EXP, bias=nm2[:, sh:sh + 1], scale=1.0,
                                accum_out=den2[:, sh:sh + 1],
                            )
                        # fixup: p *= exp(m_sh - m_glob) / den_glob, all [128,·]
                        nmg = tiny.tile([128, 1], F32, tag="nmg")
                        nc.vector.tensor_reduce(
                            out=nmg[:], in_=nm2[:],
                            axis=mybir.AxisListType.X, op=MIN,
                        )
                        dd = tiny.tile([128, 2], F32, tag="dd")
                        nc.vector.tensor_scalar(
                            out=dd[:], in0=nm2[:], scalar1=nmg[:],
                            scalar2=None, op0=SUB,
                        )
                        f2 = tiny.tile([128, 2], F32, tag="f2")
                        nc.scalar.activation(f2[:], dd[:], EXP, scale=-1.0)
                        t2 = tiny.tile([128, 2], F32, tag="t2")
                        nc.vector.tensor_mul(t2[:], den2[:], f2[:])
                        den = tiny.tile([128, 1], F32, tag="den")
                        nc.vector.tensor_add(den[:], t2[:, 0:1], t2[:, 1:2])
                        invden = tiny.tile([128, 1], F32, tag="invden")
                        nc.vector.reciprocal(invden[:], den[:])
                        sc2 = tiny.tile([128, 2], F32, tag="sc2")
                        nc.vector.tensor_scalar_mul(sc2[:], f2[:], invden[:])
                        for sh in range(2):
                            nc.vector.tensor_scalar_mul(
                                p_qt[:, sh * HS:(sh + 1) * HS],
                                p_qt[:, sh * HS:(sh + 1) * HS],
                                sc2[:, sh:sh + 1],
                            )

                        if lagged is not None:
                            emit_transposes(pt_h, *lagged)
                        lagged = (p_qt, qt)
                    emit_transposes(pt_h, *lagged)

                    # ctx^T (o-part, q-free) accumulated over s-tiles
                    ct_ps = acc_psp.tile([128, SQ], F32, tag="acc1024")
                    for qc in range(SQ // 512):
                        for st in range(ST):
                            nc.tensor.matmul(
                                ct_ps[:, qc * 512:(qc + 1) * 512],
                                v_sb[:, st, :],
                                pt_h[:, st, qc * 512:(qc + 1) * 512],
                                start=(st == 0),
                                stop=(st == ST - 1),
                            )
                    ctx_h = ctx_pool.tile([128, SQ], BF16, tag="ctx")
                    nc.scalar.copy(ctx_h[:], ct_ps[:])
                    ctxs.append(ctx_h)

                # ------- out (q-part, e-free) = sum_h ctx_h^T-slices @ Wo_h
                for qt in range(QT):
                    o_ps = acc_psp.tile([128, E], F32, tag="acc1024")
                    for h in range(H):
                        for ec in range(EC):
                            nc.tensor.matmul(
                                o_ps[:, ec * 512:(ec + 1) * 512],
                                ctxs[h][:, qt * 128:(qt + 1) * 128],
                                wo_sb[:, h, ec * 512:(ec + 1) * 512],
                                start=(h == 0),
                                stop=(h == H - 1),
                            )
                    o_sb = o_sbp.tile([128, E], F32, tag="osb")
                    nc.scalar.copy(o_sb[:], o_ps[:])
                    nc.sync.dma_start(d_out[qt * 128:(qt + 1) * 128, :], o_sb[:])

    nc.compile()
    return nc


def _split(a):
    """fp32 -> (hi, lo) bf16 pair with hi + lo ~= a."""
    hi = a.astype(ml_dtypes.bfloat16)
    lo = (a - hi.astype(np.float32)).astype(ml_dtypes.bfloat16)
    return hi, lo


def kernel(x, Wq, Wk, Wv, Wo):
    global _compiled
    from concourse.bass_utils import run_bass_kernel_spmd

    if _compiled is None:
        _compiled = _build()
    nc = _compiled

    scale = np.float32(1.0 / np.sqrt(O))
    wqh, wql = _split(Wq.astype(np.float32) * scale)
    wkh, wkl = _split(Wk.astype(np.float32))
    wvh = Wv.astype(ml_dtypes.bfloat16)
    woh = Wo.astype(ml_dtypes.bfloat16)

    in_maps = []
    for c in range(NCORES):
        b, half = divmod(c, 2)
        xT = np.ascontiguousarray(x[b].T)          # (E, S) fp32
        xh, xl = _split(xT)
        in_maps.append({
            "xkvh": xh, "xkvl": xl,
            "xqh": np.ascontiguousarray(xh[:, half * SQ:(half + 1) * SQ]),
            "xql": np.ascontiguousarray(xl[:, half * SQ:(half + 1) * SQ]),
            "wqh": wqh, "wql": wql,
            "wkh": wkh, "wkl": wkl, "wvh": wvh, "woh": woh,
        })

    res = run_bass_kernel_spmd(nc, in_maps, core_ids=list(range(NCORES)))

    out = np.empty((B, S, E), dtype=np.float32)
    for c in range(NCORES):
        b, half = divmod(c, 2)
        out[b, half * SQ:(half + 1) * SQ, :] = res.results[c]["out"]
    return out


# revision 24
# speedup vs baseline: 4.7665x; 1.0633x over previous
"""Multi-head self-attention on 8 TRN2 NeuronCores.

Problem: x(4,2048,1024), Wq(8,1024,128), Wk/Wv(1024,128), Wo(1024,1024) fp32.
out = softmax(Q K^T / sqrt(128)) V -> concat heads -> @ Wo.

Sharding: (batch, query-half) across 8 cores — core c handles batch c//2,
query rows [(c%2)*1024, (c%2)*1024+1024). K/V cover the full sequence of the
batch, so each core computes them locally from its x slice; no collectives.

Numerics: scores have std ~1024 and softmax is near-one-hot, so the
x->Q/K->scores chain needs ~fp32 precision. bf16 matmuls with hi/lo split
operands ("split3": Ah*Bh + Ah*Bl + Al*Bh, fp32 PSUM accumulation) give
~5e-6 relative matmul error at 3 cycles/row (native fp32 is 4). The x and
weight splits are precomputed on the host. V/ctx/Wo paths are plain bf16.

Layouts (partition dim first):
  xT (E,S) host-transposed; K^T (O,S) = sum_e Wk[e].T-stationary @ xT[e];
  Q_h^T (O,Sq) likewise (Wq pre-scaled by 1/sqrt(O) on host);
  scores tile (128q, 2048s) = Q^T-slice-stationary @ K^T-moving, fp32 PSUM,
  bank-chunk-major so each 512-col bank finishes early;
  softmax per q-row: per-bank DVE reduce_max -> combine(negate) -> per-bank
  ACT exp(bias=-max, accum_out=den chunk) -> den sum -> 1/den -> DVE scale;
  P transposed 128x128 via PE right after each q-tile (PE gap filler);
  ctx^T (O,Sq) = V-stationary @ P^T-moving; out (Sq,E) = ctx-slices-stationary
  @ Wo-moving (natural output layout).
"""
import numpy as np
import ml_dtypes

B, S, E, H, O = 4, 2048, 1024, 8, 128
SQ = S // 2          # query rows per core
NCORES = 8
ET = E // 128        # 8 e-tiles
ST = S // 128        # 16 s-tiles
QT = SQ // 128       # 8 q-tiles
NB = S // 512        # 4 score banks per q-tile
EC = E // 512        # 2 out-proj column chunks

_compiled = None     # cache so repeated kernel() calls skip rebuild


def _build():
    import concourse.bass as bass
    import concourse.mybir as mybir
    import concourse.tile as tile
    from concourse import bacc
    from concourse.masks import make_identity

    F32 = mybir.dt.float32
    BF16 = mybir.dt.bfloat16
    PS = bass.MemorySpace.PSUM
    EXP = mybir.ActivationFunctionType.Exp

    nc = bacc.Bacc("TRN2", target_bir_lowering=False, debug=False,
                   enable_asserts=True)

    d_xkvh = nc.dram_tensor("xkvh", (E, S), BF16, kind="ExternalInput").ap()
    d_xkvl = nc.dram_tensor("xkvl", (E, S), BF16, kind="ExternalInput").ap()
    d_xqh = nc.dram_tensor("xqh", (E, SQ), BF16, kind="ExternalInput").ap()
    d_xql = nc.dram_tensor("xql", (E, SQ), BF16, kind="ExternalInput").ap()
    d_wqh = nc.dram_tensor("wqh", (H, E, O), BF16, kind="ExternalInput").ap()
    d_wql = nc.dram_tensor("wql", (H, E, O), BF16, kind="ExternalInput").ap()
    d_wkh = nc.dram_tensor("wkh", (E, O), BF16, kind="ExternalInput").ap()
    d_wkl = nc.dram_tensor("wkl", (E, O), BF16, kind="ExternalInput").ap()
    d_wvh = nc.dram_tensor("wvh", (E, O), BF16, kind="ExternalInput").ap()
    d_woh = nc.dram_tensor("woh", (H * O, E), BF16, kind="ExternalInput").ap()
    d_out = nc.dram_tensor("out", (SQ, E), F32, kind="ExternalOutput").ap()

    with tile.TileContext(nc) as tc:
        with (
            tc.tile_pool(name="persist", bufs=1) as persist,
            tc.tile_pool(name="tiny", bufs=24) as tiny,
        ):
            ident = persist.tile([128, 128], BF16, tag="ident")
            make_identity(nc, ident[:])

            wo_sb = persist.tile([128, H, E], BF16, tag="wo")
            nc.sync.dma_start(wo_sb[:], d_woh.rearrange("(h p) e -> p h e", p=128))

            kth = persist.tile([128, S], BF16, tag="kth")
            ktl = persist.tile([128, S], BF16, tag="ktl")
            qth = persist.tile([128, H, SQ], BF16, tag="qth")
            qtl = persist.tile([128, H, SQ], BF16, tag="qtl")
            v_sb = persist.tile([128, ST, O], BF16, tag="v")

            # ---------------- prologue: K^T, V, Q^T projections ----------
            with tc.tile_pool(name="xp", bufs=1) as xp:
                wkh = xp.tile([128, ET, O], BF16, tag="wkh")
                wkl = xp.tile([128, ET, O], BF16, tag="wkl")
                nc.sync.dma_start(wkh[:], d_wkh.rearrange("(t p) o -> p t o", p=128))
                nc.sync.dma_start(wkl[:], d_wkl.rearrange("(t p) o -> p t o", p=128))
                xkvh = xp.tile([128, ET, S], BF16, tag="xkvh")
                xkvl = xp.tile([128, ET, S], BF16, tag="xkvl")
                nc.sync.dma_start(xkvh[:, 0, :], d_xkvh[0:128, :])
                nc.sync.dma_start(xkvl[:, 0, :], d_xkvl[0:128, :])
                wvh = xp.tile([128, ET, O], BF16, tag="wvh")
                nc.sync.dma_start(wvh[:], d_wvh.rearrange("(t p) o -> p t o", p=128))
                for e in range(1, ET):
                    nc.sync.dma_start(xkvh[:, e, :], d_xkvh[e * 128:(e + 1) * 128, :])
                    nc.sync.dma_start(xkvl[:, e, :], d_xkvl[e * 128:(e + 1) * 128, :])

                # K^T (128o x 2048s), split3 accumulation over e
                with tc.tile_pool(name="ktp", bufs=1, space=PS) as ktp:
                    kt_ps = ktp.tile([128, S], F32, tag="kt")
                    for e in range(ET):
                        for ti, (w, xx) in enumerate(
                            ((wkh, xkvh), (wkh, xkvl), (wkl, xkvh))
                        ):
                            for c in range(NB):
                                nc.tensor.matmul(
                                    kt_ps[:, c * 512:(c + 1) * 512],
                                    w[:, e, :],
                                    xx[:, e, c * 512:(c + 1) * 512],
                                    start=(e == 0 and ti == 0),
                                    stop=(e == ET - 1 and ti == 2),
                                )
                    nc.scalar.copy(kth[:], kt_ps[:])
                    nc.vector.tensor_sub(ktl[:], kt_ps[:], kth[:])

                # V^T (o-part) with Wv stationary (8 weight loads instead of
                # 128), then PE-transpose into the s-part tiles ctx needs
                with tc.tile_pool(name="vtp", bufs=1, space=PS) as vtp:
                    vt_ps = vtp.tile([128, S], F32, tag="vt")
                    for e in range(ET):
                        for c in range(NB):
                            nc.tensor.matmul(
                                vt_ps[:, c * 512:(c + 1) * 512],
                                wvh[:, e, :],
                                xkvh[:, e, c * 512:(c + 1) * 512],
                                start=(e == 0),
                                stop=(e == ET - 1),
                            )
                    vt_sb = xp.tile([128, S], BF16, tag="vtsb")
                    nc.scalar.copy(vt_sb[:], vt_ps[:])
                with tc.tile_pool(name="vsp", bufs=2, space=PS) as vsp:
                    for g in range(2):
                        v_st = vsp.tile([128, 8, 128], BF16, tag="vst")
                        for k in range(8):
                            st = g * 8 + k
                            nc.tensor.transpose(
                                v_st[:, k, :],
                                vt_sb[:, st * 128:(st + 1) * 128],
                                ident[:],
                            )
                        nc.vector.tensor_copy(
                            v_sb[:, g * 8:(g + 1) * 8, :], v_st[:])

                with tc.tile_pool(name="qp", bufs=3, space=PS) as qp:
                    # Q^T per head (Wq pre-scaled by 1/sqrt(O) on host)
                    xqh = xp.tile([128, ET, SQ], BF16, tag="xqh")
                    xql = xp.tile([128, ET, SQ], BF16, tag="xql")
                    for e in range(ET):
                        nc.sync.dma_start(xqh[:, e, :], d_xqh[e * 128:(e + 1) * 128, :])
                        nc.sync.dma_start(xql[:, e, :], d_xql[e * 128:(e + 1) * 128, :])
                    wqh = xp.tile([128, H, ET, O], BF16, tag="wqh")
                    wql = xp.tile([128, H, ET, O], BF16, tag="wql")
                    nc.sync.dma_start(
                        wqh[:], d_wqh.rearrange("h (t p) o -> p h t o", p=128))
                    nc.sync.dma_start(
                        wql[:], d_wql.rearrange("h (t p) o -> p h t o", p=128))

                    for h in range(H):
                        q_ps = qp.tile([128, SQ], F32, tag="qtps")
                        for e in range(ET):
                            for ti, (w, xx) in enumerate(
                                ((wqh, xqh), (wqh, xql), (wql, xqh))
                            ):
                                for c in range(SQ // 512):
                                    nc.tensor.matmul(
                                        q_ps[:, c * 512:(c + 1) * 512],
                                        w[:, h, e, :],
                                        xx[:, e, c * 512:(c + 1) * 512],
                                        start=(e == 0 and ti == 0),
                                        stop=(e == ET - 1 and ti == 2),
                                    )
                        nc.scalar.copy(qth[:, h, :], q_ps[:])
                        nc.vector.tensor_sub(qtl[:, h, :], q_ps[:], qth[:, h, :])

            # ---------------- main: per-head attention ------------------
            # PSUM budget (8 banks): "acc1024" 2-bank tiles x3 bufs shared by
            # score-halves, ctx and out accumulators (6 banks) + one 2-bank
            # transpose staging tile.  Score halves cycle through 3 slots so
            # the next q-tile's matmuls never wait on this one's softmax.
            with (
                tc.tile_pool(name="p_pool", bufs=4) as p_pool,
                tc.tile_pool(name="pt_pool", bufs=2) as pt_pool,
                tc.tile_pool(name="ctx_pool", bufs=H) as ctx_pool,
                tc.tile_pool(name="acc_ps", bufs=3, space=PS) as acc_psp,
                tc.tile_pool(name="pt_ps", bufs=1, space=PS) as pt_psp,
                tc.tile_pool(name="o_sb", bufs=2) as o_sbp,
            ):
                HS = S // 2  # 1024-wide score half

                def emit_transposes(pt_h, p_qt, qt):
                    # runs one q-tile BEHIND the softmax pipeline: all deps
                    # are long resolved, so these are always-ready PE filler
                    # and the ACT copy never stalls the exp stream
                    pt_ps = pt_psp.tile([128, ST, 128], BF16, tag="ptps")
                    for st in range(ST):
                        nc.tensor.transpose(
                            pt_ps[:, st, :],
                            p_qt[:, st * 128:(st + 1) * 128],
                            ident[:],
                        )
                    cp = nc.scalar.copy if qt % 2 else nc.vector.tensor_copy
                    cp(pt_h[:, :, qt * 128:(qt + 1) * 128], pt_ps[:])

                MIN = mybir.AluOpType.min
                SUB = mybir.AluOpType.subtract
                ctxs = []

                def emit_ctx(pt_h):
                    # ctx^T (o-part, q-free) accumulated over s-tiles; lagged
                    # into the next head's score phase as PE filler
                    ct_ps = acc_psp.tile([128, SQ], F32, tag="acc1024")
                    for qc in range(SQ // 512):
                        for st in range(ST):
                            nc.tensor.matmul(
                                ct_ps[:, qc * 512:(qc + 1) * 512],
                                v_sb[:, st, :],
                                pt_h[:, st, qc * 512:(qc + 1) * 512],
                                start=(st == 0),
                                stop=(st == ST - 1),
                            )
                    ctx_h = ctx_pool.tile([128, SQ], BF16, tag="ctx")
                    nc.scalar.copy(ctx_h[:], ct_ps[:])
                    ctxs.append(ctx_h)

                pending_ctx = None
                for h in range(H):
                    pt_h = pt_pool.tile([128, ST, SQ], BF16, tag="pt")
                    lagged = None
                    for qt in range(QT):
                        # flash-style: each half gets a LOCAL max + exp so its
                        # PSUM slot frees without waiting for the other half;
                        # tiny per-partition factors fix up the normalization.
                        nm2 = tiny.tile([128, 2], F32, tag="nm2")
                        den2 = tiny.tile([128, 2], F32, tag="den2")
                        p_qt = p_pool.tile([128, S], BF16, tag="p")
                        for sh in range(2):
                            s_ps = acc_psp.tile([128, HS], F32, tag="acc1024")
                            for ti, (qq, kk) in enumerate(
                                ((qth, kth), (qth, ktl), (qtl, kth))
                            ):
                                for c in range(2):
                                    nc.tensor.matmul(
                                        s_ps[:, c * 512:(c + 1) * 512],
                                        qq[:, h, qt * 128:(qt + 1) * 128],
                                        kk[:, sh * HS + c * 512:
                                           sh * HS + (c + 1) * 512],
                                        start=(ti == 0),
                                        stop=(ti == 2),
                                    )
                            nc.vector.reduce_max(
                                out=nm2[:, sh:sh + 1], in_=s_ps[:],
                                axis=mybir.AxisListType.X, negate=True,
                            )
                            nc.scalar.activation(
                                p_qt[:, sh * HS:(sh + 1) * HS],
                                s_ps[:],
                                EXP, bias=nm2[:, sh:sh + 1], scale=1.0,
                                accum_out=den2[:, sh:sh + 1],
                            )
                        # fixup: p *= exp(m_sh - m_glob) / den_glob, all [128,·]
                        nmg = tiny.tile([128, 1], F32, tag="nmg")
                        nc.vector.tensor_reduce(
                            out=nmg[:], in_=nm2[:],
                            axis=mybir.AxisListType.X, op=MIN,
                        )
                        dd = tiny.tile([128, 2], F32, tag="dd")
                        nc.vector.tensor_scalar(
                            out=dd[:], in0=nm2[:], scalar1=nmg[:],
                            scalar2=None, op0=SUB,
                        )
                        f2 = tiny.tile([128, 2], F32, tag="f2")
                        nc.scalar.activation(f2[:], dd[:], EXP, scale=-1.0)
                        t2 = tiny.tile([128, 2], F32, tag="t2")
                        nc.vector.tensor_mul(t2[:], den2[:], f2[:])
                        den = tiny.tile([128, 1], F32, tag="den")
                        nc.vector.tensor_add(den[:], t2[:, 0:1], t2[:, 1:2])
                        invden = tiny.tile([128, 1], F32, tag="invden")
                        nc.vector.reciprocal(invden[:], den[:])
                        sc2 = tiny.tile([128, 2], F32, tag="sc2")
                        nc.vector.tensor_scalar_mul(sc2[:], f2[:], invden[:])
                        for sh in range(2):
                            nc.vector.tensor_scalar_mul(
                                p_qt[:, sh * HS:(sh + 1) * HS],
                                p_qt[:, sh * HS:(sh + 1) * HS],
                                sc2[:, sh:sh + 1],
                            )

                        if lagged is not None:
                            emit_transposes(pt_h, *lagged)
                        lagged = (p_qt, qt)
                        if qt == 1 and pending_ctx is not None:
                            emit_ctx(pending_ctx)
                            pending_ctx = None
                    emit_transposes(pt_h, *lagged)
                    pending_ctx = pt_h
                emit_ctx(pending_ctx)

                # ------- out (q-part, e-free) = sum_h ctx_h^T-slices @ Wo_h
                for qt in range(QT):
                    o_ps = acc_psp.tile([128, E], F32, tag="acc1024")
                    for h in range(H):
                        for ec in range(EC):
                            nc.tensor.matmul(
                                o_ps[:, ec * 512:(ec + 1) * 512],
                                ctxs[h][:, qt * 128:(qt + 1) * 128],
                                wo_sb[:, h, ec * 512:(ec + 1) * 512],
                                start=(h == 0),
                                stop=(h == H - 1),
                            )
                    o_sb = o_sbp.tile([128, E], F32, tag="osb")
                    nc.scalar.copy(o_sb[:], o_ps[:])
                    nc.sync.dma_start(d_out[qt * 128:(qt + 1) * 128, :], o_sb[:])

    nc.compile()
    return nc


def _split(a):
    """fp32 -> (hi, lo) bf16 pair with hi + lo ~= a."""
    hi = a.astype(ml_dtypes.bfloat16)
    lo = (a - hi.astype(np.float32)).astype(ml_dtypes.bfloat16)
    return hi, lo


def kernel(x, Wq, Wk, Wv, Wo):
    global _compiled
    from concourse.bass_utils import run_bass_kernel_spmd

    if _compiled is None:
        _compiled = _build()
    nc = _compiled

    scale = np.float32(1.0 / np.sqrt(O))
    wqh, wql = _split(Wq.astype(np.float32) * scale)
    wkh, wkl = _split(Wk.astype(np.float32))
    wvh = Wv.astype(ml_dtypes.bfloat16)
    woh = Wo.astype(ml_dtypes.bfloat16)

    in_maps = []
    for c in range(NCORES):
        b, half = divmod(c, 2)
        xT = np.ascontiguousarray(x[b].T)          # (E, S) fp32
        xh, xl = _split(xT)
        in_maps.append({
            "xkvh": xh, "xkvl": xl,
            "xqh": np.ascontiguousarray(xh[:, half * SQ:(half + 1) * SQ]),
            "xql": np.ascontiguousarray(xl[:, half * SQ:(half + 1) * SQ]),
            "wqh": wqh, "wql": wql,
            "wkh": wkh, "wkl": wkl, "wvh": wvh, "woh": woh,
        })

    res = run_bass_kernel_spmd(nc, in_maps, core_ids=list(range(NCORES)))

    out = np.empty((B, S, E), dtype=np.float32)
    for c in range(NCORES):
        b, half = divmod(c, 2)
        out[b, half * SQ:(half + 1) * SQ, :] = res.results[c]["out"]
    return out
